# revision 17
# baseline (speedup 1.0000x reference)
"""Trainium2 Bass kernel for nn_NodeEncoder (GNN message passing + node MLP).

Strategy (no collectives):
  - Nodes are partitioned into 8 buckets of 6250 (padded to 6272 = 49*128).
  - Each core gets TWO edge lists: edges whose `row` lands in its bucket
    (for the `sent` attention call, destination index = row) and edges whose
    `col` lands in its bucket (for `recv`, destination = col).  Within a
    core, edges are grouped by destination 128-node block so that the
    segment softmax/scatter-add reduces entirely on-core via one-hot
    matmuls accumulating in PSUM.
  - Softmax max-subtraction is dropped: logits are tiny (|l| < ~1) and
    alpha = exp(l)/sum(exp(l)) is shift invariant, so we accumulate
    S = sum(e*v) and z = sum(e) per destination and divide at the end
    (out = S / (z + 1e-16)), matching the reference semantics exactly
    (including empty segments -> 0).
  - The dense 3-layer MLP (with LayerNorm) runs node-parallel per bucket.
  - Host (numpy) work is limited to: slicing/permuting inputs per core,
    integer index/metadata construction, and dtype casts for staging.

Feature-major layout ([feature, edge]) is used for the per-edge matmuls
(W on the stationary port, up-to-512-wide moving operand); edge-major
tiles (via PE transposes) are used for the one-hot scatter matmuls.
"""

import os
os.environ.setdefault("BASS_DISABLE_FRAME_TO_TRACEBACK", "1")

import numpy as np
import ml_dtypes

import concourse.bass as bass
import concourse.bacc as bacc
import concourse.mybir as mybir
import concourse.tile as tile
from concourse.bass_utils import run_bass_kernel_spmd
from concourse.masks import make_identity

# ---------------------------------------------------------------- constants
NN = 50000          # nodes
NE = 400000         # edges
LAT = 128           # latent
HID = 512           # hidden
NH = 2              # heads
HD = 64             # head dim
NG = 500            # graphs
NCORES = 8
BUCK = NN // NCORES             # 6250 real nodes per core
NBLK = (BUCK + 127) // 128      # 49 blocks of 128 nodes
BUCKP = NBLK * 128              # 6272 padded nodes per core
CBLK = 1152                     # edge-slot capacity per 128-node block
G = CBLK // 128                 # 9 groups of 128 edges per block
ECAP = NBLK * CBLK              # 56448 edge slots per core per ordering
NGRP = NBLK * G                 # 441 groups total
LN_EPS = 1e-5
SUBS = [(0, 512), (512, 512), (1024, 128)]  # block split into <=512 chunks
assert sum(s for _, s in SUBS) == CBLK

F32 = mybir.dt.float32
BF16 = mybir.dt.bfloat16
I32 = mybir.dt.int32
AF = mybir.ActivationFunctionType
ALU = mybir.AluOpType
AX = mybir.AxisListType

BF = ml_dtypes.bfloat16

_NC_CACHE = {}


# ------------------------------------------------------------- device build
def build_nc(debug=False):
    nc = bacc.Bacc(None)

    # -------- I/O declarations (order = NEFF parameter order)
    nf16 = nc.declare_dram_parameter("nf16", [NN, LAT], BF16, isOutput=False)
    nfT16 = nc.declare_dram_parameter("nfT16", [LAT, BUCKP], BF16, isOutput=False)
    glT16 = nc.declare_dram_parameter("glT16", [LAT, BUCKP], BF16, isOutput=False)
    ea_h = {}
    qi_h = {}
    ki_h = {}
    dl_h = {}
    for o in ("s", "r"):
        ea_h[o] = nc.declare_dram_parameter(f"eaT_{o}", [LAT, ECAP], BF16, isOutput=False)
        qi_h[o] = nc.declare_dram_parameter(f"qiT_{o}", [128, NGRP], I32, isOutput=False)
        ki_h[o] = nc.declare_dram_parameter(f"kiT_{o}", [128, NGRP], I32, isOutput=False)
        dl_h[o] = nc.declare_dram_parameter(f"dlT_{o}", [128, NGRP], F32, isOutput=False)
    w1_h = nc.declare_dram_parameter("w1", [3 * LAT, LAT], BF16, isOutput=False)
    w2_h = nc.declare_dram_parameter("w2p", [LAT, NH], BF16, isOutput=False)
    w3_h = nc.declare_dram_parameter("w3", [2 * LAT, LAT], BF16, isOutput=False)
    b1_h = nc.declare_dram_parameter("b1c", [LAT, 1], F32, isOutput=False)
    b3_h = nc.declare_dram_parameter("b3c", [LAT, 1], F32, isOutput=False)
    mw0_h = nc.declare_dram_parameter("mw0", [HID, HID], BF16, isOutput=False)
    mw1_h = nc.declare_dram_parameter("mw1", [HID, HID], BF16, isOutput=False)
    mw2_h = nc.declare_dram_parameter("mw2", [HID, LAT], BF16, isOutput=False)
    mr_h = nc.declare_dram_parameter("mrows", [1, 6 * HID], F32, isOutput=False)
    mb2_h = nc.declare_dram_parameter("mb2r", [1, LAT], F32, isOutput=False)
    out_h = nc.declare_dram_parameter("out", [BUCKP, LAT], F32, isOutput=True)
    if debug:
        dS_h = {o: nc.declare_dram_parameter(f"dS_{o}", [128, NBLK * 130], F32,
                                             isOutput=True) for o in ("s", "r")}
        dnm_h = nc.declare_dram_parameter("dnm", [BUCKP, 2 * LAT], BF16,
                                          isOutput=True)
        dh1_h = nc.declare_dram_parameter("dh1", [BUCKP, HID], BF16,
                                          isOutput=True)

    with tile.TileContext(nc) as tc:
        with tc.tile_pool(name="persist", bufs=1) as pp:
            # ---- constants
            iota_i = pp.tile([128, 128], I32)
            nc.gpsimd.iota(iota_i[:], pattern=[[1, 128]], base=0, channel_multiplier=0)
            iota_f = pp.tile([128, 128], F32)
            nc.vector.tensor_copy(iota_f[:], iota_i[:])
            id16 = pp.tile([128, 128], BF16)
            make_identity(nc, id16[:])
            ones1 = pp.tile([1, 128], F32)
            nc.vector.memset(ones1[:], 1.0)

            # ---- weights to SBUF
            w1sb = pp.tile([128, 3 * LAT], BF16)
            for k in range(3):
                nc.sync.dma_start(w1sb[:, k * 128:(k + 1) * 128],
                                  w1_h[k * 128:(k + 1) * 128, :])
            w2sb = pp.tile([128, NH], BF16)
            nc.sync.dma_start(w2sb[:], w2_h[:, :])
            w3sb = pp.tile([128, 2 * LAT], BF16)
            for k in range(2):
                nc.sync.dma_start(w3sb[:, k * 128:(k + 1) * 128],
                                  w3_h[k * 128:(k + 1) * 128, :])
            b1sb = pp.tile([128, 1], F32)
            nc.sync.dma_start(b1sb[:], b1_h[:, :])
            b3sb = pp.tile([128, 1], F32)
            nc.sync.dma_start(b3sb[:], b3_h[:, :])
            mw0sb = pp.tile([128, 4 * HID], BF16)
            mw1sb = pp.tile([128, 4 * HID], BF16)
            for k in range(4):
                nc.sync.dma_start(mw0sb[:, k * HID:(k + 1) * HID],
                                  mw0_h[k * 128:(k + 1) * 128, :])
                nc.sync.dma_start(mw1sb[:, k * HID:(k + 1) * HID],
                                  mw1_h[k * 128:(k + 1) * 128, :])
            mw2sb = pp.tile([128, 4 * LAT], BF16)
            for k in range(4):
                nc.sync.dma_start(mw2sb[:, k * LAT:(k + 1) * LAT],
                                  mw2_h[k * 128:(k + 1) * 128, :])
            mrsb = pp.tile([1, 6 * HID], F32)
            nc.sync.dma_start(mrsb[:], mr_h[:, :])
            mb2sb = pp.tile([1, LAT], F32)
            nc.sync.dma_start(mb2sb[:], mb2_h[:, :])

            # ---- broadcast LN-row constants into [128, HID] tiles
            bc = []  # MB0, LS0, LB0, MB1, LS1, LB1
            with tc.tile_pool(name="bc_ps", bufs=1, space="PSUM") as bps:
                for i in range(6):
                    t = pp.tile([128, HID], F32, tag=f"bc{i}")
                    ps = bps.tile([128, HID], F32, tag="bps")
                    nc.tensor.matmul(ps[:], ones1[:],
                                     mrsb[0:1, i * HID:(i + 1) * HID],
                                     start=True, stop=True)
                    nc.vector.tensor_copy(t[:], ps[:])
                    bc.append(t)
                mb2bc = pp.tile([128, LAT], F32)
                ps = bps.tile([128, LAT], F32, tag="bps")
                nc.tensor.matmul(ps[:], ones1[:], mb2sb[:, :], start=True, stop=True)
                nc.vector.tensor_copy(mb2bc[:], ps[:])

            # ---- per-destination accumulators (S|z per head), per ordering
            S_sb = {
                "s": pp.tile([128, NBLK * 130], F32, tag="Ss", name="S_s"),
                "r": pp.tile([128, NBLK * 130], F32, tag="Sr", name="S_r"),
            }

            # ================= edge phase =================
            with (
                tc.tile_pool(name="ep", bufs=3) as ep,
                tc.tile_pool(name="gp", bufs=4) as gp,
                tc.tile_pool(name="sp", bufs=3) as sp,
                tc.tile_pool(name="mp", bufs=2) as mp,
                tc.tile_pool(name="ps_S", bufs=1, space="PSUM") as ps_S,
                tc.tile_pool(name="ps_hp", bufs=1, space="PSUM") as ps_hp,
                tc.tile_pool(name="ps_vp", bufs=1, space="PSUM") as ps_vp,
                tc.tile_pool(name="ps_lg", bufs=1, space="PSUM") as ps_lg,
                tc.tile_pool(name="ps_tq", bufs=2, space="PSUM") as ps_tq,
                tc.tile_pool(name="ps_ve", bufs=1, space="PSUM") as ps_ve,
            ):
                for o in ("s", "r"):
                    for b in range(NBLK):
                        qi = mp.tile([128, G], I32, tag="qi")
                        ki = mp.tile([128, G], I32, tag="ki")
                        dl = mp.tile([128, G], F32, tag="dl")
                        nc.sync.dma_start(qi[:], qi_h[o][:, b * G:(b + 1) * G])
                        nc.sync.dma_start(ki[:], ki_h[o][:, b * G:(b + 1) * G])
                        nc.sync.dma_start(dl[:], dl_h[o][:, b * G:(b + 1) * G])
                        S0_ps = ps_S.tile([128, 65], F32, tag="S0")
                        S1_ps = ps_S.tile([128, 65], F32, tag="S1")
                        for off, sz in SUBS:
                            g0 = off // 128
                            ng = sz // 128
                            ea_t = ep.tile([128, sz], BF16, tag="ea")
                            nc.sync.dma_start(
                                ea_t[:], ea_h[o][:, b * CBLK + off: b * CBLK + off + sz])
                            tq = ps_tq.tile([128, sz], BF16, tag="tqk")
                            tk = ps_tq.tile([128, sz], BF16, tag="tqk")
                            for g in range(ng):
                                gq = gp.tile([128, 128], BF16, tag="gq")
                                nc.gpsimd.indirect_dma_start(
                                    out=gq[:], out_offset=None, in_=nf16[:, :],
                                    in_offset=bass.IndirectOffsetOnAxis(
                                        ap=qi[:, g0 + g:g0 + g + 1], axis=0))
                                nc.tensor.transpose(
                                    tq[:, g * 128:(g + 1) * 128], gq[:], id16[:])
                                gk = gp.tile([128, 128], BF16, tag="gk")
                                nc.gpsimd.indirect_dma_start(
                                    out=gk[:], out_offset=None, in_=nf16[:, :],
                                    in_offset=bass.IndirectOffsetOnAxis(
                                        ap=ki[:, g0 + g:g0 + g + 1], axis=0))
                                nc.tensor.transpose(
                                    tk[:, g * 128:(g + 1) * 128], gk[:], id16[:])
                            qT = ep.tile([128, sz], BF16, tag="qT")
                            kT = ep.tile([128, sz], BF16, tag="kT")
                            nc.scalar.copy(qT[:], tq[:])
                            nc.scalar.copy(kT[:], tk[:])
                            # hpre^T = W1q.T q^T + W1k.T k^T + W1e.T ea^T
                            hp = ps_hp.tile([128, sz], F32, tag="hp")
                            nc.tensor.matmul(hp[:], w1sb[:, 0:128], qT[:],
                                             start=True, stop=False)
                            nc.tensor.matmul(hp[:], w1sb[:, 128:256], kT[:],
                                             start=False, stop=False)
                            nc.tensor.matmul(hp[:], w1sb[:, 256:384], ea_t[:],
                                             start=False, stop=True)
                            hT = ep.tile([128, sz], BF16, tag="hT")
                            nc.scalar.activation(hT[:], hp[:], AF.Lrelu,
                                                 bias=b1sb[:, 0:1], alpha=0.2)
                            lg = ps_lg.tile([NH, sz], F32, tag="lg")
                            nc.tensor.matmul(lg[:], w2sb[:], hT[:],
                                             start=True, stop=True)
                            eT = ep.tile([NH, sz], BF16, tag="eT")
                            nc.scalar.activation(eT[:], lg[:], AF.Exp)
                            # v^T = W3k.T k^T + W3e.T ea^T (+ b3)
                            vp = ps_vp.tile([128, sz], F32, tag="vp")
                            nc.tensor.matmul(vp[:], w3sb[:, 0:128], kT[:],
                                             start=True, stop=False)
                            nc.tensor.matmul(vp[:], w3sb[:, 128:256], ea_t[:],
                                             start=False, stop=True)
                            vT = ep.tile([128, sz], BF16, tag="vT")
                            nc.scalar.activation(vT[:], vp[:], AF.Identity,
                                                 bias=b3sb[:, 0:1])
                            # scatter per 128-edge group
                            for g in range(ng):
                                gg = g0 + g
                                sl = slice(g * 128, (g + 1) * 128)
                                ve = ps_ve.tile([128, 130], BF16, tag="ve")
                                nc.tensor.transpose(ve[:, 0:128], vT[:, sl], id16[:])
                                nc.tensor.transpose(ve[:, 128:130], eT[:, sl],
                                                    id16[:NH, :NH])
                                ecs = sp.tile([128, NH], F32, tag="ecs")
                                nc.vector.tensor_copy(ecs[:], ve[:, 128:130])
                                V0 = sp.tile([128, 65], BF16, tag="V0")
                                V1 = sp.tile([128, 65], BF16, tag="V1")
                                nc.vector.tensor_copy(V0[:, 0:64], ve[:, 0:64])
                                nc.vector.memset(V0[:, 64:65], 1.0)
                                nc.vector.tensor_copy(V1[:, 0:64], ve[:, 64:128])
                                nc.vector.memset(V1[:, 64:65], 1.0)
                                P0 = sp.tile([128, 128], BF16, tag="P0")
                                P1 = sp.tile([128, 128], BF16, tag="P1")
                                nc.vector.tensor_scalar(
                                    P0[:], iota_f[:], dl[:, gg:gg + 1], ecs[:, 0:1],
                                    ALU.is_equal, ALU.mult)
                                nc.vector.tensor_scalar(
                                    P1[:], iota_f[:], dl[:, gg:gg + 1], ecs[:, 1:2],
                                    ALU.is_equal, ALU.mult)
                                nc.tensor.matmul(S0_ps[:], P0[:], V0[:],
                                                 start=(gg == 0), stop=(gg == G - 1))
                                nc.tensor.matmul(S1_ps[:], P1[:], V1[:],
                                                 start=(gg == 0), stop=(gg == G - 1))
                        nc.vector.tensor_copy(
                            S_sb[o][:, b * 130:b * 130 + 65], S0_ps[:])
                        nc.vector.tensor_copy(
                            S_sb[o][:, b * 130 + 65:(b + 1) * 130], S1_ps[:])

            if debug:
                for o in ("s", "r"):
                    nc.sync.dma_start(dS_h[o][:, :], S_sb[o][:])

            # ================= MLP phase =================
            def ln_relu(mpool, psum_in, MB, LS, LB):
                t0 = mpool.tile([128, HID], F32, tag="t0")
                nc.vector.tensor_tensor(t0[:], psum_in[:], MB[:], op=ALU.add)
                mu = mpool.tile([128, 1], F32, tag="mu")
                nc.vector.reduce_sum(out=mu[:], in_=t0[:], axis=AX.X)
                mus = mpool.tile([128, 1], F32, tag="mus")
                nc.vector.tensor_scalar_mul(mus[:], mu[:], 1.0 / HID)
                xc = mpool.tile([128, HID], F32, tag="xc")
                nc.vector.tensor_scalar(xc[:], t0[:], mus[:, 0:1], None,
                                        ALU.subtract)
                sq = mpool.tile([128, HID], F32, tag="sq")
                var = mpool.tile([128, 1], F32, tag="var")
                nc.scalar.activation(sq[:], xc[:], AF.Square,
                                     accum_out=var[:, 0:1])
                var2 = mpool.tile([128, 1], F32, tag="var2")
                nc.vector.tensor_scalar(var2[:], var[:], 1.0 / HID, LN_EPS,
                                        ALU.mult, ALU.add)
                sd = mpool.tile([128, 1], F32, tag="sd")
                nc.scalar.sqrt(sd[:], var2[:])
                rs = mpool.tile([128, 1], F32, tag="rs")
                nc.vector.reciprocal(rs[:], sd[:])
                xn = mpool.tile([128, HID], F32, tag="xn")
                nc.scalar.activation(xn[:], xc[:], AF.Identity, bias=0.0,
                                     scale=rs[:, 0:1])
                g1 = mpool.tile([128, HID], F32, tag="g1")
                nc.vector.tensor_tensor(g1[:], xn[:], LS[:], op=ALU.mult)
                g2 = mpool.tile([128, HID], F32, tag="g2")
                nc.vector.tensor_tensor(g2[:], g1[:], LB[:], op=ALU.add)
                h = mpool.tile([128, HID], BF16, tag="h")
                nc.scalar.activation(h[:], g2[:], AF.Relu)
                return h

            with (
                tc.tile_pool(name="ml", bufs=2) as ml,
                tc.tile_pool(name="ps_mp", bufs=2, space="PSUM") as ps_mp,
                tc.tile_pool(name="ps_mt", bufs=2, space="PSUM") as ps_mt,
                tc.tile_pool(name="ps_mo", bufs=2, space="PSUM") as ps_mo,
            ):
                for b in range(NBLK):
                    sides = {}
                    for o in ("s", "r"):
                        base = b * 130
                        rz0 = ml.tile([128, 1], F32, tag="rz0")
                        nc.vector.tensor_scalar_add(
                            rz0[:], S_sb[o][:, base + 64:base + 65], 1e-16)
                        nc.vector.reciprocal(rz0[:], rz0[:])
                        rz1 = ml.tile([128, 1], F32, tag="rz1")
                        nc.vector.tensor_scalar_add(
                            rz1[:], S_sb[o][:, base + 129:base + 130], 1e-16)
                        nc.vector.reciprocal(rz1[:], rz1[:])
                        nm = ml.tile([128, 128], BF16, tag="nm")
                        nc.vector.tensor_scalar(
                            nm[:, 0:64], S_sb[o][:, base:base + 64],
                            rz0[:, 0:1], None, ALU.mult)
                        nc.vector.tensor_scalar(
                            nm[:, 64:128], S_sb[o][:, base + 65:base + 129],
                            rz1[:, 0:1], None, ALU.mult)
                        tp = ps_mt.tile([128, 128], BF16, tag="mt")
                        nc.tensor.transpose(tp[:], nm[:], id16[:])
                        sT = ml.tile([128, 128], BF16, tag=f"{o}T")
                        nc.scalar.copy(sT[:], tp[:])
                        sides[o] = sT
                        if debug:
                            co = 0 if o == "s" else LAT
                            nc.sync.dma_start(
                                dnm_h[b * 128:(b + 1) * 128, co:co + LAT], nm[:])
                    nfb = ml.tile([128, 128], BF16, tag="nfb")
                    nc.sync.dma_start(nfb[:], nfT16[:, b * 128:(b + 1) * 128])
                    glb = ml.tile([128, 128], BF16, tag="glb")
                    nc.sync.dma_start(glb[:], glT16[:, b * 128:(b + 1) * 128])
                    chunks = [nfb, sides["s"], sides["r"], glb]
                    h1p = ps_mp.tile([128, HID], F32, tag="mp")
                    for i, ch in enumerate(chunks):
                        nc.tensor.matmul(h1p[:], ch[:],
                                         mw0sb[:, i * HID:(i + 1) * HID],
                                         start=(i == 0), stop=(i == 3))
                    h1 = ln_relu(ml, h1p, bc[0], bc[1], bc[2])
                    if debug:
                        nc.sync.dma_start(dh1_h[b * 128:(b + 1) * 128, :], h1[:])
                    h1t = ps_mt.tile([128, HID], BF16, tag="mt4")
                    for c in range(4):
                        nc.tensor.transpose(h1t[:, c * 128:(c + 1) * 128],
                                            h1[:, c * 128:(c + 1) * 128], id16[:])
                    h1T = ml.tile([128, HID], BF16, tag="h1T")
                    nc.scalar.copy(h1T[:], h1t[:])
                    h2p = ps_mp.tile([128, HID], F32, tag="mp")
                    for i in range(4):
                        nc.tensor.matmul(h2p[:], h1T[:, i * 128:(i + 1) * 128],
                                         mw1sb[:, i * HID:(i + 1) * HID],
                                         start=(i == 0), stop=(i == 3))
                    h2 = ln_relu(ml, h2p, bc[3], bc[4], bc[5])
                    h2t = ps_mt.tile([128, HID], BF16, tag="mt4")
                    for c in range(4):
                        nc.tensor.transpose(h2t[:, c * 128:(c + 1) * 128],
                                            h2[:, c * 128:(c + 1) * 128], id16[:])
                    h2T = ml.tile([128, HID], BF16, tag="h2T")
                    nc.scalar.copy(h2T[:], h2t[:])
                    op = ps_mo.tile([128, LAT], F32, tag="mo")
                    for i in range(4):
                        nc.tensor.matmul(op[:], h2T[:, i * 128:(i + 1) * 128],
                                         mw2sb[:, i * LAT:(i + 1) * LAT],
                                         start=(i == 0), stop=(i == 3))
                    ot = ml.tile([128, LAT], F32, tag="ot")
                    nc.vector.tensor_tensor(ot[:], op[:], mb2bc[:], op=ALU.add)
                    nc.sync.dma_start(out_h[b * 128:(b + 1) * 128, :], ot[:])

    nc.finalize()
    return nc


# ------------------------------------------------------------- host packing
def pack_ordering(dst, src, edge_attr, core):
    """Pack one core's edge list for one ordering (dst = destination index).

    Returns eaT [128, ECAP] bf16, qiT/kiT [128, NGRP] i32, dlT [128, NGRP] f32.
    """
    lo = core * BUCK
    sel = np.flatnonzero((dst >= lo) & (dst < lo + BUCK))
    ldst = (dst[sel] - lo).astype(np.int64)
    blk = ldst >> 7
    order = np.argsort(blk, kind="stable")
    sel = sel[order]
    ldst = ldst[order]
    blk = blk[order]
    cnt = np.bincount(blk, minlength=NBLK)
    if cnt.max() > CBLK:
        raise RuntimeError(f"block capacity {CBLK} exceeded: {cnt.max()}")
    starts = np.zeros(NBLK, np.int64)
    starts[1:] = np.cumsum(cnt)[:-1]
    slot = blk * CBLK + (np.arange(len(sel)) - starts[blk])

    qidx = np.zeros(ECAP, np.int32)
    kidx = np.zeros(ECAP, np.int32)
    dloc = np.full(ECAP, -1.0, np.float32)
    qidx[slot] = dst[sel].astype(np.int32)
    kidx[slot] = src[sel].astype(np.int32)
    dloc[slot] = (ldst & 127).astype(np.float32)

    eaT = np.zeros((128, ECAP), BF)
    eaT[:, slot] = edge_attr[sel].T.astype(BF)

    qiT = np.ascontiguousarray(qidx.reshape(NGRP, 128).T)
    kiT = np.ascontiguousarray(kidx.reshape(NGRP, 128).T)
    dlT = np.ascontiguousarray(dloc.reshape(NGRP, 128).T)
    return eaT, qiT, kiT, dlT


def pack_inputs(edges, node_feat, edge_attr, u, num_nodes,
                w1, b1, w2, w3, b3,
                mw0, mb0, ls0, lb0, mw1, mb1, ls1, lb1, mw2, mb2):
    row, col = np.asarray(edges[0]), np.asarray(edges[1])
    node_feat = np.asarray(node_feat, np.float32)
    edge_attr = np.asarray(edge_attr, np.float32)

    # glob = repeat(u, num_nodes) clipped/padded to NN (matches jnp.repeat
    # with total_repeat_length: overflow indices clamp to the last row)
    reps = np.asarray(num_nodes, np.int64)
    gids = np.repeat(np.arange(NG), reps)
    if len(gids) >= NN:
        gids = gids[:NN]
    else:
        gids = np.concatenate([gids, np.full(NN - len(gids), NG - 1, np.int64)])
    glob = np.asarray(u, np.float32)[gids]  # [NN, LAT]

    w2p = np.zeros((LAT, NH), np.float32)
    for h in range(NH):
        w2p[h * HD:(h + 1) * HD, h] = np.asarray(w2)[h]

    mrows = np.stack([mb0, ls0, lb0, mb1, ls1, lb1]).astype(np.float32).reshape(1, -1)

    common = {
        "nf16": node_feat.astype(BF),
        "w1": np.asarray(w1, np.float32).astype(BF),
        "w2p": w2p.astype(BF),
        "w3": np.asarray(w3, np.float32).astype(BF),
        "b1c": np.asarray(b1, np.float32).reshape(LAT, 1),
        "b3c": np.asarray(b3, np.float32).reshape(LAT, 1),
        "mw0": np.asarray(mw0, np.float32).astype(BF),
        "mw1": np.asarray(mw1, np.float32).astype(BF),
        "mw2": np.asarray(mw2, np.float32).astype(BF),
        "mrows": mrows,
        "mb2r": np.asarray(mb2, np.float32).reshape(1, LAT),
    }

    in_maps = []
    for c in range(NCORES):
        m = dict(common)
        lo = c * BUCK
        nfT = np.zeros((LAT, BUCKP), BF)
        nfT[:, :BUCK] = node_feat[lo:lo + BUCK].T.astype(BF)
        glT = np.zeros((LAT, BUCKP), BF)
        glT[:, :BUCK] = glob[lo:lo + BUCK].T.astype(BF)
        m["nfT16"] = nfT
        m["glT16"] = glT
        for o, (d, s) in (("s", (row, col)), ("r", (col, row))):
            eaT, qiT, kiT, dlT = pack_ordering(d, s, edge_attr, c)
            m[f"eaT_{o}"] = eaT
            m[f"qiT_{o}"] = qiT
            m[f"kiT_{o}"] = kiT
            m[f"dlT_{o}"] = dlT
        in_maps.append(m)
    return in_maps


# ------------------------------------------------------------------- driver
def kernel(**inputs):
    in_maps = pack_inputs(**inputs)
    if "nc" not in _NC_CACHE:
        _NC_CACHE["nc"] = build_nc()
    nc = _NC_CACHE["nc"]
    res = run_bass_kernel_spmd(nc, in_maps, list(range(NCORES))).results
    out = np.empty((NN, LAT), np.float32)
    for c in range(NCORES):
        out[c * BUCK:(c + 1) * BUCK] = res[c]["out"][:BUCK]
    return out
